# revision 20
# baseline (speedup 1.0000x reference)
"""Trainium2 Bass kernel for the CSPN fusion module (nn_CSPNFusion).

Sharding: the 384-row image is split across 8 NeuronCores (48 rows each,
1-row halo).  Per core:
  - The three convs (64->9 guide, 64->1 gview, 64->1 gating) run fused as a
    single 64->11 conv: 9 accumulated tap-matmuls, two views packed into
    K=128 with a block-diagonal weight matrix, 4 output rows concurrent in 4
    PE column groups (tile_position col tiling), N=384 per matmul.
  - PSUM is evacuated by ScalarE to SBUF, dumped to a DRAM scratch, and
    re-loaded in a [row-partitions, plane, W] layout where every pointwise op
    runs with ~100 active partitions.
  - All transcendentals use the {exp, ln} table set: sigmoid(x) =
    exp(-ln(1+exp(-x))), 1/x = exp(-ln(x)).
  - The 6 CSPN iterations collapse to the closed form
        m = base * (1-gc^6)/(1-gc) + gc^6 * proj
    with gc clamped just below 1 for fp32 stability.
  - Engine ops never shift partitions (hardware requirement); the vertical
    conv taps for `base` use three row-shifted copies of the projected
    planes, and the cross-view folds bounce through DRAM.
"""
import numpy as np

import concourse.bacc as bacc
import concourse.tile as tile
from concourse import mybir
from concourse.bass_utils import run_bass_kernel_spmd

F32 = mybir.dt.float32
BF16 = mybir.dt.bfloat16
AF = mybir.ActivationFunctionType
ALU = mybir.AluOpType

NCORES = 8
N, V, FCH, C, H, W = 2, 4, 64, 3, 384, 384
RS = H // NCORES          # 48 output rows per core
RH = RS + 2               # 50 rows incl halo
WP = W + 2                # padded width
NPAIR = 4                 # view pairs (2 views each)
CHUNK = 12                # output rows per X chunk
NCHUNK = RS // CHUNK
UCLAMP = 0.99999905

_CACHE = {}


def _build(with_bias):
    nc = bacc.Bacc("TRN2", target_bir_lowering=False, debug=False,
                   num_devices=NCORES)
    gf = nc.declare_dram_parameter("gf", [2 * NPAIR, FCH, RH, W], F32, isOutput=False)
    pr = nc.declare_dram_parameter("pr", [2 * NPAIR, C, RH, W], BF16, isOutput=False)
    mk = nc.declare_dram_parameter("mk", [2 * NPAIR, RS, W], BF16, isOutput=False)
    dp = nc.declare_dram_parameter("dp", [N, C, RS, W], F32, isOutput=False)
    wp = nc.declare_dram_parameter("wp", [128, 9, 32], F32, isOutput=False)
    if with_bias:
        bp = nc.declare_dram_parameter("bp", [1, 32], F32, isOutput=False)
    o0 = nc.declare_dram_parameter("o0", [N, C, RS, W], F32, isOutput=True)
    o1 = nc.declare_dram_parameter("o1", [N, C, RS, W], F32, isOutput=True)

    import contextlib
    with tile.TileContext(nc) as tc, contextlib.ExitStack() as ctx:
        kpool = ctx.enter_context(tc.tile_pool(name="kpool", bufs=1))
        apool = ctx.enter_context(tc.tile_pool(name="apool", bufs=1))
        xpool = ctx.enter_context(tc.tile_pool(name="xpool", bufs=2))
        plpool = ctx.enter_context(tc.tile_pool(name="plpool", bufs=2))
        prpool = ctx.enter_context(tc.tile_pool(name="prpool", bufs=1))
        vpool = ctx.enter_context(tc.tile_pool(name="vpool", bufs=1))
        stgpool = ctx.enter_context(tc.tile_pool(name="stgpool", bufs=4))
        pspool = ctx.enter_context(tc.tile_pool(name="pspool", bufs=6, space="PSUM"))
        dpool = ctx.enter_context(tc.tile_pool(name="dpool", bufs=6, space="DRAM"))
        d1pool = ctx.enter_context(tc.tile_pool(name="d1pool", bufs=1, space="DRAM"))

        wt = kpool.tile([128, 9, 32], F32)
        ceps = kpool.tile([128, 1], F32)
        nc.vector.memset(ceps, 1e-7)
        nc.sync.dma_start(out=wt, in_=wp[:])
        if with_bias:
            bt = kpool.tile([1, 32], F32)
            nc.sync.dma_start(out=bt, in_=bp[:])
            ones = kpool.tile([1, W], F32)
            nc.vector.memset(ones, 1.0)

        # deep image [2n x 50, 3, W]; rows live at partitions h*50 + (1..48)
        DP = apool.tile([100, C, W], F32)
        nc.scalar.memzero(DP)
        for n in range(N):
            nc.sync.dma_start(
                out=DP.rearrange("(n p) c w -> n p c w", n=2)[n, 1:49],
                in_=dp[n].rearrange("c r w -> r c w"),
            )
        D2 = apool.tile([100, C, W], F32)   # 2*deep - 1
        nc.vector.tensor_scalar(out=D2, in0=DP, scalar1=2.0, scalar2=-1.0,
                                op0=ALU.mult, op1=ALU.add)

        # per-n accumulators in [2v-half x 50] layout
        accN = [apool.tile([100, C, W], BF16, name=f"accN{i}") for i in range(N)]
        accD = [apool.tile([100, W], BF16, name=f"accD{i}") for i in range(N)]
        accG = [apool.tile([100, W], F32, name=f"accG{i}") for i in range(N)]
        accM = [apool.tile([100, W], BF16, name=f"accM{i}") for i in range(N)]

        for pair in range(NPAIR):
            nn_ = pair // 2
            first = (pair % 2 == 0)

            # projected planes, 3 row-shifted copies: PRD[:, d, c, :] holds
            # proj row (p%50 - 1) + (d - 1) at partition p.
            PRD = prpool.tile([100, 3, C, WP], BF16)
            nc.scalar.memzero(PRD)
            for d_ in range(3):
                nc.vector.memset(PRD[:, d_, :, 0:1], 0.0)
                nc.vector.memset(PRD[:, d_, :, WP - 1:WP], 0.0)
            for hh in range(2):
                for d in range(3):
                    nc.sync.dma_start(
                        out=PRD.rearrange("(h p) d c w -> h p d c w", h=2)
                               [hh, 1:49, d, :, 1:385],
                        in_=pr[2 * pair + hh].rearrange("c r w -> r c w")[d:d + 48],
                    )
            MK = prpool.tile([100, W], BF16)
            nc.scalar.memzero(MK)
            for hh in range(2):
                nc.sync.dma_start(
                    out=MK.rearrange("(v p) w -> v p w", v=2)[hh, 1:49],
                    in_=mk[2 * pair + hh],
                )

            PLf = plpool.tile([100, 12, W], F32, name="PLf")
            PL = PLf[:, 0:11]
            nc.scalar.memzero(PLf)
            EB = plpool.tile([100, 10, W], BF16, name="EB")
            nc.scalar.memzero(EB)

            for chk in range(NCHUNK):
                X = xpool.tile([128, CHUNK + 3, WP], F32, name="X")[:, 0:CHUNK + 2]
                nc.vector.memset(X[:, :, 0:1], 0.0)
                nc.vector.memset(X[:, :, WP - 1:WP], 0.0)
                for vv in range(2):
                    nc.sync.dma_start(
                        out=X.rearrange("(v c) r w -> v c r w", v=2)[vv, :, :, 1:385],
                        in_=gf[2 * pair + vv, :,
                               chk * CHUNK:chk * CHUNK + CHUNK + 2],
                    )
                for rg in range(CHUNK // 4):
                    ps = pspool.tile([128, W], F32)
                    if with_bias:
                        for cg in range(4):
                            nc.tensor.matmul(ps[32 * cg:32 * cg + 32], bt, ones,
                                             start=True, stop=False,
                                             tile_position=(0, 32 * cg),
                                             skip_group_check=True)
                    for tap in range(9):
                        di, dj = tap // 3, tap % 3
                        for cg in range(4):
                            row = rg * 4 + cg + di
                            nc.tensor.matmul(
                                ps[32 * cg:32 * cg + 32],
                                wt[:, tap],
                                X[:, row, dj:dj + W],
                                start=(tap == 0 and not with_bias),
                                stop=(tap == 8),
                                tile_position=(0, 32 * cg),
                                         skip_group_check=True,
                            )
                    stg = stgpool.tile([128, W], F32)
                    nc.scalar.copy(out=stg, in_=ps)
                    scr = dpool.tile([128, W], F32)
                    nc.sync.dma_start(out=scr, in_=stg)
                    rb = 1 + chk * CHUNK + rg * 4
                    for hh in range(2):
                        nc.sync.dma_start(
                            out=PL.rearrange("(h p) k w -> h p k w", h=2)[hh, rb:rb + 4],
                            in_=scr.rearrange("(g x) w -> g x w", g=4)
                                   [:, 11 * hh:11 * hh + 11],
                        )

            # ---- pointwise (all ops partition-aligned on [0:100]) ----
            nc.scalar.activation(out=EB[:, 0:9], in_=PL[:, 0:9], func=AF.Exp)
            t4 = vpool.tile([100, 4, W], BF16)
            nc.vector.tensor_add(t4, EB[:, 0:4], EB[:, 4:8])
            t2 = vpool.tile([100, 2, W], BF16)
            nc.vector.tensor_add(t2, t4[:, 0:2], t4[:, 2:4])
            S = vpool.tile([100, W], BF16)
            nc.vector.tensor_add(S, t2[:, 0], t2[:, 1])
            nc.vector.tensor_add(S, S, EB[:, 8])
            rf = vpool.tile([100, W], F32)
            nc.scalar.activation(out=rf, in_=S, func=AF.Ln)
            nc.scalar.activation(out=rf, in_=rf, func=AF.Exp, scale=-1.0)
            r = vpool.tile([100, W], BF16)
            nc.vector.tensor_copy(out=r, in_=rf)
            gv = vpool.tile([100, W], BF16)
            nc.scalar.activation(out=gv, in_=PL[:, 9], func=AF.Exp, scale=-1.0)
            nc.scalar.activation(out=gv, in_=gv, func=AF.Ln, bias=1.0)
            nc.scalar.activation(out=gv, in_=gv, func=AF.Exp, scale=-1.0)
            nc.vector.tensor_mul(gv, gv, MK)

            B = vpool.tile([100, C, W], BF16)
            tmpb = vpool.tile([100, C, W], BF16)
            firstk = True
            for tap in range(9):
                if tap == 4:
                    continue
                di, dj = tap // 3, tap % 3
                ek = EB[:, tap:tap + 1].broadcast_to((100, C, W))
                prs = PRD[:, di, :, dj:dj + W]
                if firstk:
                    nc.vector.tensor_mul(B, ek, prs)
                    firstk = False
                else:
                    nc.vector.tensor_mul(tmpb, ek, prs)
                    nc.vector.tensor_add(B, B, tmpb)

            u = vpool.tile([100, W], F32)
            nc.vector.tensor_mul(u, EB[:, 4], r)
            nc.vector.tensor_scalar_min(out=u, in0=u, scalar1=UCLAMP)
            u2 = vpool.tile([100, W], F32)
            nc.vector.tensor_mul(u2, u, u)
            u4 = vpool.tile([100, W], F32)
            nc.vector.tensor_mul(u4, u2, u2)
            u6 = vpool.tile([100, W], F32)
            nc.vector.tensor_mul(u6, u2, u4)
            wm = vpool.tile([100, W], F32)
            nc.vector.tensor_scalar(out=wm, in0=u, scalar1=-1.0, scalar2=1.0,
                                    op0=ALU.mult, op1=ALU.add)
            iw = vpool.tile([100, W], F32)
            nc.scalar.activation(out=iw, in_=wm, func=AF.Ln)
            nc.scalar.activation(out=iw, in_=iw, func=AF.Exp, scale=-1.0)
            T = vpool.tile([100, W], F32)
            nc.vector.tensor_scalar(out=T, in0=u6, scalar1=-1.0, scalar2=1.0,
                                    op0=ALU.mult, op1=ALU.add)
            nc.vector.tensor_mul(T, T, iw)
            rT = vpool.tile([100, W], F32)
            nc.vector.tensor_mul(rT, r, T)

            m = vpool.tile([100, C, W], BF16)
            nc.vector.tensor_mul(m, B, rT.unsqueeze(1).broadcast_to((100, C, W)))
            u6b = vpool.tile([100, W], BF16)
            nc.vector.tensor_copy(out=u6b, in_=u6)
            nc.vector.tensor_mul(tmpb, u6b.unsqueeze(1).broadcast_to((100, C, W)),
                                 PRD[:, 1, :, 1:1 + W])
            nc.vector.tensor_add(m, m, tmpb)
            nc.vector.tensor_mul(m, m, gv.unsqueeze(1).broadcast_to((100, C, W)))

            if first:
                nc.vector.tensor_copy(out=accN[nn_], in_=m)
                nc.vector.tensor_copy(out=accD[nn_], in_=gv)
                nc.vector.tensor_copy(out=accG[nn_], in_=PL[:, 10])
                nc.vector.tensor_copy(out=accM[nn_], in_=MK)
            else:
                nc.vector.tensor_add(accN[nn_], accN[nn_], m)
                nc.vector.tensor_add(accD[nn_], accD[nn_], gv)
                nc.vector.tensor_add(accG[nn_], accG[nn_], PL[:, 10])
                nc.vector.tensor_add(accM[nn_], accM[nn_], MK)

        # ---- fold view halves via DRAM repack into [2n x 50] layout ----
        sN = d1pool.tile([2, 2, 50, C, W], BF16)
        sD = d1pool.tile([2, 2, 50, W], BF16)
        sG = d1pool.tile([2, 2, 50, W], F32)
        sM = d1pool.tile([2, 2, 50, W], BF16)
        for n in range(N):
            for hh in range(2):
                nc.sync.dma_start(out=sN[hh, n], in_=accN[n][50 * hh:50 * hh + 50])
                nc.sync.dma_start(out=sD[hh, n], in_=accD[n][50 * hh:50 * hh + 50])
                nc.sync.dma_start(out=sG[hh, n], in_=accG[n][50 * hh:50 * hh + 50])
                nc.sync.dma_start(out=sM[hh, n], in_=accM[n][50 * hh:50 * hh + 50])

        TA = vpool.tile([100, C, W], BF16, tag="m")
        TB = vpool.tile([100, C, W], BF16, tag="tmpb")
        nc.sync.dma_start(out=TA, in_=sN[0])
        nc.sync.dma_start(out=TB, in_=sN[1])
        FN = vpool.tile([100, C, W], BF16, tag="B")
        nc.vector.tensor_add(FN, TA, TB)

        TD0 = vpool.tile([100, W], BF16, tag="u")
        TD1 = vpool.tile([100, W], BF16, tag="u2")
        nc.sync.dma_start(out=TD0, in_=sD[0])
        nc.sync.dma_start(out=TD1, in_=sD[1])
        FD = vpool.tile([100, W], BF16, tag="u4")
        nc.vector.tensor_add(FD, TD0, TD1)

        TG0 = vpool.tile([100, W], F32, tag="u6")
        TG1 = vpool.tile([100, W], F32, tag="wm")
        nc.sync.dma_start(out=TG0, in_=sG[0])
        nc.sync.dma_start(out=TG1, in_=sG[1])
        FG = vpool.tile([100, W], F32, tag="iw")
        nc.vector.tensor_add(FG, TG0, TG1)

        TM0 = vpool.tile([100, W], BF16, tag="T")
        TM1 = vpool.tile([100, W], BF16, tag="rT")
        nc.sync.dma_start(out=TM0, in_=sM[0])
        nc.sync.dma_start(out=TM1, in_=sM[1])
        FM = vpool.tile([100, W], BF16, tag="S")
        nc.vector.tensor_add(FM, TM0, TM1)

        # gating = sigmoid(FG/4) * (FM > 0)
        g = vpool.tile([100, W], F32, tag="r")
        nc.scalar.activation(out=g, in_=FG, func=AF.Exp, scale=-0.25)
        nc.scalar.activation(out=g, in_=g, func=AF.Ln, bias=1.0)
        nc.scalar.activation(out=g, in_=g, func=AF.Exp, scale=-1.0)
        pm = vpool.tile([100, W], F32, tag="pmf")
        nc.vector.tensor_scalar(out=pm, in0=FM, scalar1=1e30, scalar2=1.0,
                                op0=ALU.mult, op1=ALU.min)
        nc.vector.tensor_mul(g, g, pm)

        rden = vpool.tile([100, W], BF16, tag="rdenb")
        nc.scalar.activation(out=rden, in_=FD, func=AF.Ln, bias=ceps[0:100])
        nc.scalar.activation(out=rden, in_=rden, func=AF.Exp, scale=-1.0)

        fused = vpool.tile([100, C, W], F32, tag="m")
        nc.vector.tensor_mul(fused, FN,
                             rden.unsqueeze(1).broadcast_to((100, C, W)))
        o1t = apool.tile([100, C, W], F32)
        nc.vector.tensor_scalar(out=o1t, in0=fused, scalar1=2.0, scalar2=-1.0,
                                op0=ALU.mult, op1=ALU.add)
        o0t = apool.tile([100, C, W], F32)
        nc.vector.tensor_sub(o0t, fused, DP)
        nc.vector.tensor_mul(o0t, o0t, g.unsqueeze(1).broadcast_to((100, C, W)))
        nc.vector.scalar_tensor_tensor(out=o0t, in0=o0t, scalar=2.0, in1=D2,
                                       op0=ALU.mult, op1=ALU.add)

        for n in range(N):
            nc.sync.dma_start(
                out=o0[n].rearrange("c r w -> r c w"),
                in_=o0t.rearrange("(n p) c w -> n p c w", n=2)[n, 1:49],
            )
            nc.sync.dma_start(
                out=o1[n].rearrange("c r w -> r c w"),
                in_=o1t.rearrange("(n p) c w -> n p c w", n=2)[n, 1:49],
            )

    nc.compile()
    return nc


def _prep(inputs):
    gfv = np.asarray(inputs["guidance_feats"], np.float32)
    dpv = np.asarray(inputs["deep_image"], np.float32)
    prv = np.asarray(inputs["projected"], np.float32)
    mkv = np.asarray(inputs["projected_masks"], np.float32)

    gw = np.asarray(inputs["gating_w"], np.float32)
    gs = np.asarray(inputs["gating_s"], np.float32)
    gb = np.asarray(inputs["gating_b"], np.float32)
    vw = np.asarray(inputs["gview_w"], np.float32)
    vs = np.asarray(inputs["gview_s"], np.float32)
    vb = np.asarray(inputs["gview_b"], np.float32)
    uw = np.asarray(inputs["guide_w"], np.float32)
    us = np.asarray(inputs["guide_s"], np.float32)
    ub = np.asarray(inputs["guide_b"], np.float32)

    w11 = np.concatenate([uw * us[:, None, None, None],
                          vw * vs[:, None, None, None],
                          gw * gs[:, None, None, None]], axis=0)  # [11,64,3,3]
    b11 = np.concatenate([ub, vb, gb]).astype(np.float32)
    wpv = np.zeros((128, 9, 32), np.float32)
    for t in range(9):
        di, dj = t // 3, t % 3
        wpv[0:64, t, 0:11] = w11[:, :, di, dj].T
        wpv[64:128, t, 11:22] = w11[:, :, di, dj].T
    with_bias = bool(np.any(b11 != 0.0))
    bpv = np.zeros((1, 32), np.float32)
    bpv[0, 0:11] = b11
    bpv[0, 11:22] = b11

    import ml_dtypes
    gfp = np.pad(gfv, ((0, 0), (0, 0), (1, 1), (0, 0)))
    prp = np.pad(prv, ((0, 0), (0, 0), (1, 1), (0, 0))).astype(ml_dtypes.bfloat16)
    mkv = mkv.astype(ml_dtypes.bfloat16)

    in_maps = []
    for cidx in range(NCORES):
        r0 = cidx * RS
        mmap = {
            "gf": np.ascontiguousarray(gfp[:, :, r0:r0 + RH]),
            "pr": np.ascontiguousarray(prp[:, :, r0:r0 + RH]),
            "mk": np.ascontiguousarray(mkv[:, 0, r0:r0 + RS]),
            "dp": np.ascontiguousarray(dpv[:, :, r0:r0 + RS]),
            "wp": wpv,
        }
        if with_bias:
            mmap["bp"] = bpv
        in_maps.append(mmap)
    return in_maps, with_bias


_EXEC = {}


def _get_exec(with_bias):
    """Build (once) a cached jitted 8-core executable for the program."""
    if with_bias in _EXEC:
        return _EXEC[with_bias]
    if with_bias not in _CACHE:
        _CACHE[with_bias] = _build(with_bias)
    nc = _CACHE[with_bias]

    import jax
    from jax.experimental.shard_map import shard_map
    from jax.sharding import Mesh, PartitionSpec
    from concourse import bass2jax
    from concourse import mybir as mb

    bass2jax.install_neuronx_cc_hook()
    part_name = nc.partition_id_tensor.name if nc.partition_id_tensor else None
    in_names, out_names, out_avals, zero_shapes = [], [], [], []
    for alloc in nc.m.functions[0].allocations:
        if not isinstance(alloc, mb.MemoryLocationSet):
            continue
        name = alloc.memorylocations[0].name
        if alloc.kind == "ExternalInput":
            if name == part_name:
                continue
            in_names.append(name)
        elif alloc.kind == "ExternalOutput":
            out_names.append(name)
            shape = tuple(alloc.tensor_shape)
            dtype = mb.dt.np(alloc.dtype)
            out_avals.append(jax.core.ShapedArray(shape, dtype))
            zero_shapes.append((shape, dtype))
    n_params = len(in_names)
    all_in_names = in_names + out_names
    if part_name is not None:
        all_in_names = all_in_names + [part_name]

    def _body(*args):
        operands = list(args)
        if part_name is not None:
            operands.append(bass2jax.partition_id_tensor())
        outs = bass2jax._bass_exec_p.bind(
            *operands,
            out_avals=tuple(out_avals),
            in_names=tuple(all_in_names),
            out_names=tuple(out_names),
            lowering_input_output_aliases=(),
            sim_require_finite=True,
            sim_require_nnan=True,
            nc=nc,
        )
        return tuple(outs)

    mesh = Mesh(np.asarray(jax.devices()[:NCORES]), ("core",))
    n_outs = len(out_names)
    fn = jax.jit(
        shard_map(_body, mesh=mesh,
                  in_specs=(PartitionSpec("core"),) * (n_params + n_outs),
                  out_specs=(PartitionSpec("core"),) * n_outs,
                  check_rep=False),
        keep_unused=True,
    )
    info = {
        "fn": fn, "in_names": in_names, "out_names": out_names,
        "zero_shapes": zero_shapes, "mesh": mesh,
    }
    _EXEC[with_bias] = info
    return info


def _concat_inputs(info, in_maps):
    return [np.concatenate([m[name] for m in in_maps], axis=0)
            for name in info["in_names"]]


def _zeros(info):
    return [np.zeros((NCORES * s[0],) + tuple(s[1:]), d)
            for s, d in info["zero_shapes"]]


def kernel(**inputs):
    in_maps, with_bias = _prep(inputs)
    info = _get_exec(with_bias)
    out_arrs = info["fn"](*_concat_inputs(info, in_maps), *_zeros(info))
    outs = {}
    for i, name in enumerate(info["out_names"]):
        a = np.asarray(out_arrs[i])
        outs[name] = a.reshape(NCORES, N, C, RS, W)
    out0 = np.concatenate([outs["o0"][c] for c in range(NCORES)], axis=2)
    out1 = np.concatenate([outs["o1"][c] for c in range(NCORES)], axis=2)
    return out0, out1


def time_exec_ns(inputs, reps_a=4, reps_b=16):
    """Marginal per-execution time with device-resident inputs (slope method)."""
    import time as _time
    import jax
    from jax.sharding import NamedSharding, PartitionSpec

    in_maps, with_bias = _prep(inputs)
    info = _get_exec(with_bias)
    sh = NamedSharding(info["mesh"], PartitionSpec("core"))
    dev_in = [jax.device_put(x, sh) for x in _concat_inputs(info, in_maps)]
    dev_z = [jax.device_put(z, sh) for z in _zeros(info)]
    fn = info["fn"]

    out = fn(*dev_in, *dev_z)
    jax.block_until_ready(out)

    def run(k):
        t0 = _time.perf_counter()
        o = None
        for _ in range(k):
            o = fn(*dev_in, *dev_z)
        jax.block_until_ready(o)
        return _time.perf_counter() - t0

    run(2)
    ta = min(run(reps_a) for _ in range(3))
    tb = min(run(reps_b) for _ in range(3))
    slope = (tb - ta) / (reps_b - reps_a)
    return slope * 1e9


def time_exec_loop_ns(inputs, iters=24):
    """On-device timing: run the NEFF `iters` times inside one jax.lax.scan
    dispatch, so per-call host/tunnel overhead is excluded."""
    import time as _time
    import jax
    import jax.numpy as jnp
    from jax.experimental.shard_map import shard_map
    from jax.sharding import Mesh, PartitionSpec, NamedSharding
    from concourse import bass2jax
    from concourse import mybir as mb

    in_maps, with_bias = _prep(inputs)
    info = _get_exec(with_bias)
    nc = _CACHE[with_bias]
    mesh = info["mesh"]
    sh = NamedSharding(mesh, PartitionSpec("core"))
    dev_in = [jax.device_put(x, sh) for x in _concat_inputs(info, in_maps)]
    dev_z = [jax.device_put(z, sh) for z in _zeros(info)]

    part_name = nc.partition_id_tensor.name if nc.partition_id_tensor else None
    in_names = info["in_names"]
    out_names = info["out_names"]
    out_avals = []
    for alloc in nc.m.functions[0].allocations:
        if isinstance(alloc, mb.MemoryLocationSet) and alloc.kind == "ExternalOutput":
            out_avals.append(jax.core.ShapedArray(tuple(alloc.tensor_shape),
                                                  mb.dt.np(alloc.dtype)))
    all_in_names = list(in_names) + list(out_names)
    if part_name is not None:
        all_in_names.append(part_name)

    def _body(*args):
        operands = list(args)
        if part_name is not None:
            operands.append(bass2jax.partition_id_tensor())
        return tuple(bass2jax._bass_exec_p.bind(
            *operands,
            out_avals=tuple(out_avals),
            in_names=tuple(all_in_names),
            out_names=tuple(out_names),
            lowering_input_output_aliases=(),
            sim_require_finite=True,
            sim_require_nnan=True,
            nc=nc,
        ))

    smb = shard_map(_body, mesh=mesh,
                    in_specs=(PartitionSpec("core"),) * (len(in_names) + len(out_names)),
                    out_specs=(PartitionSpec("core"),) * len(out_names),
                    check_rep=False)

    def make_loop(k):
        def loop(ins, zs):
            def step(c, _):
                outs = smb(*ins, *zs)
                return c + outs[0].ravel()[0], None
            c, _ = jax.lax.scan(step, jnp.float32(0.0), None, length=k)
            return c
        return jax.jit(loop, static_argnums=())

    f1 = make_loop(2)
    f2 = make_loop(2 + iters)
    r = f1(dev_in, dev_z); jax.block_until_ready(r)
    r = f2(dev_in, dev_z); jax.block_until_ready(r)

    def run(f):
        t0 = _time.perf_counter()
        jax.block_until_ready(f(dev_in, dev_z))
        return _time.perf_counter() - t0

    t1 = min(run(f1) for _ in range(3))
    t2 = min(run(f2) for _ in range(3))
    return (t2 - t1) / iters * 1e9


# revision 22
# speedup vs baseline: 1.1692x; 1.1692x over previous
"""Trainium2 Bass kernel for the CSPN fusion module (nn_CSPNFusion).

Sharding: the 384-row image is split across 8 NeuronCores (48 rows each,
1-row halo).  Per core:
  - The three convs (64->9 guide, 64->1 gview, 64->1 gating) run fused as a
    single 64->11 conv: 9 accumulated tap-matmuls, two views packed into
    K=128 with a block-diagonal weight matrix, 4 output rows concurrent in 4
    PE column groups (tile_position col tiling), N=384 per matmul.
  - PSUM is evacuated by ScalarE to SBUF, dumped to a DRAM scratch, and
    re-loaded in a [row-partitions, plane, W] layout where every pointwise op
    runs with ~100 active partitions.
  - All transcendentals use the {exp, ln} table set: sigmoid(x) =
    exp(-ln(1+exp(-x))), 1/x = exp(-ln(x)).
  - The 6 CSPN iterations collapse to the closed form
        m = base * (1-gc^6)/(1-gc) + gc^6 * proj
    with gc clamped just below 1 for fp32 stability.
  - Engine ops never shift partitions (hardware requirement); the vertical
    conv taps for `base` use three row-shifted copies of the projected
    planes, and the cross-view folds bounce through DRAM.
"""
import numpy as np

import concourse.bacc as bacc
import concourse.tile as tile
from concourse import mybir
from concourse.bass_utils import run_bass_kernel_spmd

F32 = mybir.dt.float32
BF16 = mybir.dt.bfloat16
AF = mybir.ActivationFunctionType
ALU = mybir.AluOpType

NCORES = 8
N, V, FCH, C, H, W = 2, 4, 64, 3, 384, 384
RS = H // NCORES          # 48 output rows per core
RH = RS + 2               # 50 rows incl halo
WP = W + 2                # padded width
NPAIR = 4                 # view pairs (2 views each)
CHUNK = 12                # output rows per X chunk
NCHUNK = RS // CHUNK
UCLAMP = 0.99999905

_CACHE = {}


def _build(with_bias):
    nc = bacc.Bacc("TRN2", target_bir_lowering=False, debug=False,
                   num_devices=NCORES)
    gf = nc.declare_dram_parameter("gf", [2 * NPAIR, FCH, RH, W], F32, isOutput=False)
    pr = nc.declare_dram_parameter("pr", [2 * NPAIR, C, RH, W], F32, isOutput=False)
    mk = nc.declare_dram_parameter("mk", [2 * NPAIR, RS, W], F32, isOutput=False)
    dp = nc.declare_dram_parameter("dp", [N, C, RS, W], F32, isOutput=False)
    wp = nc.declare_dram_parameter("wp", [128, 9, 32], F32, isOutput=False)
    if with_bias:
        bp = nc.declare_dram_parameter("bp", [1, 32], F32, isOutput=False)
    o0 = nc.declare_dram_parameter("o0", [N, C, RS, W], F32, isOutput=True)
    o1 = nc.declare_dram_parameter("o1", [N, C, RS, W], F32, isOutput=True)

    import contextlib
    with tile.TileContext(nc) as tc, contextlib.ExitStack() as ctx:
        kpool = ctx.enter_context(tc.tile_pool(name="kpool", bufs=1))
        apool = ctx.enter_context(tc.tile_pool(name="apool", bufs=1))
        xpool = ctx.enter_context(tc.tile_pool(name="xpool", bufs=2))
        plpool = ctx.enter_context(tc.tile_pool(name="plpool", bufs=2))
        prpool = ctx.enter_context(tc.tile_pool(name="prpool", bufs=1))
        vpool = ctx.enter_context(tc.tile_pool(name="vpool", bufs=1))
        stgpool = ctx.enter_context(tc.tile_pool(name="stgpool", bufs=4))
        pspool = ctx.enter_context(tc.tile_pool(name="pspool", bufs=6, space="PSUM"))
        dpool = ctx.enter_context(tc.tile_pool(name="dpool", bufs=6, space="DRAM"))
        d1pool = ctx.enter_context(tc.tile_pool(name="d1pool", bufs=1, space="DRAM"))

        wt = kpool.tile([128, 9, 32], F32)
        ceps = kpool.tile([128, 1], F32)
        nc.vector.memset(ceps, 1e-7)
        nc.sync.dma_start(out=wt, in_=wp[:])
        if with_bias:
            bt = kpool.tile([1, 32], F32)
            nc.sync.dma_start(out=bt, in_=bp[:])
            ones = kpool.tile([1, W], F32)
            nc.vector.memset(ones, 1.0)

        # deep image [2n x 50, 3, W]; rows live at partitions h*50 + (1..48)
        DP = apool.tile([100, C, W], F32)
        nc.scalar.memzero(DP)
        for n in range(N):
            nc.sync.dma_start(
                out=DP.rearrange("(n p) c w -> n p c w", n=2)[n, 1:49],
                in_=dp[n].rearrange("c r w -> r c w"),
            )
        D2 = apool.tile([100, C, W], F32)   # 2*deep - 1
        nc.vector.tensor_scalar(out=D2, in0=DP, scalar1=2.0, scalar2=-1.0,
                                op0=ALU.mult, op1=ALU.add)

        # per-n accumulators in [2v-half x 50] layout
        accN = [apool.tile([100, C, W], F32, name=f"accN{i}") for i in range(N)]
        accD = [apool.tile([100, W], F32, name=f"accD{i}") for i in range(N)]
        accG = [apool.tile([100, W], F32, name=f"accG{i}") for i in range(N)]
        accM = [apool.tile([100, W], F32, name=f"accM{i}") for i in range(N)]

        for pair in range(NPAIR):
            nn_ = pair // 2
            first = (pair % 2 == 0)

            # projected planes, 3 row-shifted copies: PRD[:, d, c, :] holds
            # proj row (p%50 - 1) + (d - 1) at partition p.
            PRD = prpool.tile([100, 3, C, WP], F32)
            nc.scalar.memzero(PRD)
            for d_ in range(3):
                nc.vector.memset(PRD[:, d_, :, 0:1], 0.0)
                nc.vector.memset(PRD[:, d_, :, WP - 1:WP], 0.0)
            for hh in range(2):
                for d in range(3):
                    nc.sync.dma_start(
                        out=PRD.rearrange("(h p) d c w -> h p d c w", h=2)
                               [hh, 1:49, d, :, 1:385],
                        in_=pr[2 * pair + hh].rearrange("c r w -> r c w")[d:d + 48],
                    )
            MK = prpool.tile([100, W], F32)
            nc.scalar.memzero(MK)
            for hh in range(2):
                nc.sync.dma_start(
                    out=MK.rearrange("(v p) w -> v p w", v=2)[hh, 1:49],
                    in_=mk[2 * pair + hh],
                )

            PLf = plpool.tile([100, 12, W], F32, name="PLf")
            PL = PLf[:, 0:11]
            nc.scalar.memzero(PLf)

            for chk in range(NCHUNK):
                X = xpool.tile([128, CHUNK + 3, WP], F32, name="X")[:, 0:CHUNK + 2]
                nc.vector.memset(X[:, :, 0:1], 0.0)
                nc.vector.memset(X[:, :, WP - 1:WP], 0.0)
                for vv in range(2):
                    nc.sync.dma_start(
                        out=X.rearrange("(v c) r w -> v c r w", v=2)[vv, :, :, 1:385],
                        in_=gf[2 * pair + vv, :,
                               chk * CHUNK:chk * CHUNK + CHUNK + 2],
                    )
                for rg in range(CHUNK // 4):
                    ps = pspool.tile([128, W], F32)
                    if with_bias:
                        for cg in range(4):
                            nc.tensor.matmul(ps[32 * cg:32 * cg + 32], bt, ones,
                                             start=True, stop=False,
                                             tile_position=(0, 32 * cg),
                                             skip_group_check=True)
                    for tap in range(9):
                        di, dj = tap // 3, tap % 3
                        for cg in range(4):
                            row = rg * 4 + cg + di
                            nc.tensor.matmul(
                                ps[32 * cg:32 * cg + 32],
                                wt[:, tap],
                                X[:, row, dj:dj + W],
                                start=(tap == 0 and not with_bias),
                                stop=(tap == 8),
                                tile_position=(0, 32 * cg),
                                         skip_group_check=True,
                            )
                    stg = stgpool.tile([128, W], F32)
                    nc.scalar.copy(out=stg, in_=ps)
                    scr = dpool.tile([128, W], F32)
                    nc.sync.dma_start(out=scr, in_=stg)
                    rb = 1 + chk * CHUNK + rg * 4
                    for hh in range(2):
                        nc.sync.dma_start(
                            out=PL.rearrange("(h p) k w -> h p k w", h=2)[hh, rb:rb + 4],
                            in_=scr.rearrange("(g x) w -> g x w", g=4)
                                   [:, 11 * hh:11 * hh + 11],
                        )

            # ---- pointwise (all ops partition-aligned on [0:100]) ----
            nc.scalar.activation(out=PL[:, 0:9], in_=PL[:, 0:9], func=AF.Exp)
            t4 = vpool.tile([100, 4, W], F32)
            nc.vector.tensor_add(t4, PL[:, 0:4], PL[:, 4:8])
            t2 = vpool.tile([100, 2, W], F32)
            nc.vector.tensor_add(t2, t4[:, 0:2], t4[:, 2:4])
            S = vpool.tile([100, W], F32)
            nc.vector.tensor_add(S, t2[:, 0], t2[:, 1])
            nc.vector.tensor_add(S, S, PL[:, 8])
            r = vpool.tile([100, W], F32)
            nc.scalar.activation(out=r, in_=S, func=AF.Ln)
            nc.scalar.activation(out=r, in_=r, func=AF.Exp, scale=-1.0)
            gv = vpool.tile([100, W], F32)
            nc.scalar.activation(out=gv, in_=PL[:, 9], func=AF.Exp, scale=-1.0)
            nc.scalar.activation(out=gv, in_=gv, func=AF.Ln, bias=1.0)
            nc.scalar.activation(out=gv, in_=gv, func=AF.Exp, scale=-1.0)
            nc.vector.tensor_mul(gv, gv, MK)

            # base: taps 0-3 on DVE into B, taps 5-8 on GPSIMD into Bg
            B = vpool.tile([100, C, W], F32)
            tmpb = vpool.tile([100, C, W], F32)
            Bg = vpool.tile([100, C, W], F32)
            tmpg = vpool.tile([100, C, W], F32)
            fd = True
            fg = True
            for tap in range(9):
                if tap == 4:
                    continue
                di, dj = tap // 3, tap % 3
                ek = PL[:, tap:tap + 1].broadcast_to((100, C, W))
                prs = PRD[:, di, :, dj:dj + W]
                if tap < 4:
                    if fd:
                        nc.vector.tensor_mul(B, ek, prs)
                        fd = False
                    else:
                        nc.vector.tensor_mul(tmpb, ek, prs)
                        nc.vector.tensor_add(B, B, tmpb)
                else:
                    if fg:
                        nc.gpsimd.tensor_mul(Bg, ek, prs)
                        fg = False
                    else:
                        nc.gpsimd.tensor_mul(tmpg, ek, prs)
                        nc.gpsimd.tensor_add(Bg, Bg, tmpg)
            nc.vector.tensor_add(B, B, Bg)

            u = vpool.tile([100, W], F32)
            nc.vector.tensor_mul(u, PL[:, 4], r)
            nc.vector.tensor_scalar_min(out=u, in0=u, scalar1=UCLAMP)
            u2 = vpool.tile([100, W], F32)
            nc.vector.tensor_mul(u2, u, u)
            u4 = vpool.tile([100, W], F32)
            nc.vector.tensor_mul(u4, u2, u2)
            u6 = vpool.tile([100, W], F32)
            nc.vector.tensor_mul(u6, u2, u4)
            wm = vpool.tile([100, W], F32)
            nc.vector.tensor_scalar(out=wm, in0=u, scalar1=-1.0, scalar2=1.0,
                                    op0=ALU.mult, op1=ALU.add)
            iw = vpool.tile([100, W], F32)
            nc.scalar.activation(out=iw, in_=wm, func=AF.Ln)
            nc.scalar.activation(out=iw, in_=iw, func=AF.Exp, scale=-1.0)
            T = vpool.tile([100, W], F32)
            nc.vector.tensor_scalar(out=T, in0=u6, scalar1=-1.0, scalar2=1.0,
                                    op0=ALU.mult, op1=ALU.add)
            nc.vector.tensor_mul(T, T, iw)
            rT = vpool.tile([100, W], F32)
            nc.vector.tensor_mul(rT, r, T)

            m = vpool.tile([100, C, W], F32)
            nc.vector.tensor_mul(m, B, rT.unsqueeze(1).broadcast_to((100, C, W)))
            nc.vector.tensor_mul(tmpb, u6.unsqueeze(1).broadcast_to((100, C, W)),
                                 PRD[:, 1, :, 1:1 + W])
            nc.vector.tensor_add(m, m, tmpb)
            nc.vector.tensor_mul(m, m, gv.unsqueeze(1).broadcast_to((100, C, W)))

            if first:
                nc.vector.tensor_copy(out=accN[nn_], in_=m)
                nc.vector.tensor_copy(out=accD[nn_], in_=gv)
                nc.vector.tensor_copy(out=accG[nn_], in_=PL[:, 10])
                nc.vector.tensor_copy(out=accM[nn_], in_=MK)
            else:
                nc.vector.tensor_add(accN[nn_], accN[nn_], m)
                nc.vector.tensor_add(accD[nn_], accD[nn_], gv)
                nc.vector.tensor_add(accG[nn_], accG[nn_], PL[:, 10])
                nc.vector.tensor_add(accM[nn_], accM[nn_], MK)

        # ---- fold view halves via DRAM repack into [2n x 50] layout ----
        sN = d1pool.tile([2, 2, 50, C, W], F32)
        sD = d1pool.tile([2, 2, 50, W], F32)
        sG = d1pool.tile([2, 2, 50, W], F32)
        sM = d1pool.tile([2, 2, 50, W], F32)
        for n in range(N):
            for hh in range(2):
                nc.sync.dma_start(out=sN[hh, n], in_=accN[n][50 * hh:50 * hh + 50])
                nc.sync.dma_start(out=sD[hh, n], in_=accD[n][50 * hh:50 * hh + 50])
                nc.sync.dma_start(out=sG[hh, n], in_=accG[n][50 * hh:50 * hh + 50])
                nc.sync.dma_start(out=sM[hh, n], in_=accM[n][50 * hh:50 * hh + 50])

        TA = vpool.tile([100, C, W], F32, tag="m")
        TB = vpool.tile([100, C, W], F32, tag="tmpb")
        nc.sync.dma_start(out=TA, in_=sN[0])
        nc.sync.dma_start(out=TB, in_=sN[1])
        FN = vpool.tile([100, C, W], F32, tag="B")
        nc.vector.tensor_add(FN, TA, TB)

        TD0 = vpool.tile([100, W], F32, tag="u")
        TD1 = vpool.tile([100, W], F32, tag="u2")
        nc.sync.dma_start(out=TD0, in_=sD[0])
        nc.sync.dma_start(out=TD1, in_=sD[1])
        FD = vpool.tile([100, W], F32, tag="u4")
        nc.vector.tensor_add(FD, TD0, TD1)

        TG0 = vpool.tile([100, W], F32, tag="u6")
        TG1 = vpool.tile([100, W], F32, tag="wm")
        nc.sync.dma_start(out=TG0, in_=sG[0])
        nc.sync.dma_start(out=TG1, in_=sG[1])
        FG = vpool.tile([100, W], F32, tag="iw")
        nc.vector.tensor_add(FG, TG0, TG1)

        TM0 = vpool.tile([100, W], F32, tag="T")
        TM1 = vpool.tile([100, W], F32, tag="rT")
        nc.sync.dma_start(out=TM0, in_=sM[0])
        nc.sync.dma_start(out=TM1, in_=sM[1])
        FM = vpool.tile([100, W], F32, tag="S")
        nc.vector.tensor_add(FM, TM0, TM1)

        # gating = sigmoid(FG/4) * (FM > 0)
        g = vpool.tile([100, W], F32, tag="r")
        nc.scalar.activation(out=g, in_=FG, func=AF.Exp, scale=-0.25)
        nc.scalar.activation(out=g, in_=g, func=AF.Ln, bias=1.0)
        nc.scalar.activation(out=g, in_=g, func=AF.Exp, scale=-1.0)
        pm = vpool.tile([100, W], F32, tag="pmf")
        nc.vector.tensor_scalar(out=pm, in0=FM, scalar1=1e30, scalar2=1.0,
                                op0=ALU.mult, op1=ALU.min)
        nc.vector.tensor_mul(g, g, pm)

        rden = vpool.tile([100, W], F32, tag="u")
        nc.scalar.activation(out=rden, in_=FD, func=AF.Ln, bias=ceps[0:100])
        nc.scalar.activation(out=rden, in_=rden, func=AF.Exp, scale=-1.0)

        fused = vpool.tile([100, C, W], F32, tag="m")
        nc.vector.tensor_mul(fused, FN,
                             rden.unsqueeze(1).broadcast_to((100, C, W)))
        o1t = apool.tile([100, C, W], F32)
        nc.vector.tensor_scalar(out=o1t, in0=fused, scalar1=2.0, scalar2=-1.0,
                                op0=ALU.mult, op1=ALU.add)
        o0t = apool.tile([100, C, W], F32)
        nc.vector.tensor_sub(o0t, fused, DP)
        nc.vector.tensor_mul(o0t, o0t, g.unsqueeze(1).broadcast_to((100, C, W)))
        nc.vector.scalar_tensor_tensor(out=o0t, in0=o0t, scalar=2.0, in1=D2,
                                       op0=ALU.mult, op1=ALU.add)

        for n in range(N):
            nc.sync.dma_start(
                out=o0[n].rearrange("c r w -> r c w"),
                in_=o0t.rearrange("(n p) c w -> n p c w", n=2)[n, 1:49],
            )
            nc.sync.dma_start(
                out=o1[n].rearrange("c r w -> r c w"),
                in_=o1t.rearrange("(n p) c w -> n p c w", n=2)[n, 1:49],
            )

    nc.compile()
    return nc


def _prep(inputs):
    gfv = np.asarray(inputs["guidance_feats"], np.float32)
    dpv = np.asarray(inputs["deep_image"], np.float32)
    prv = np.asarray(inputs["projected"], np.float32)
    mkv = np.asarray(inputs["projected_masks"], np.float32)

    gw = np.asarray(inputs["gating_w"], np.float32)
    gs = np.asarray(inputs["gating_s"], np.float32)
    gb = np.asarray(inputs["gating_b"], np.float32)
    vw = np.asarray(inputs["gview_w"], np.float32)
    vs = np.asarray(inputs["gview_s"], np.float32)
    vb = np.asarray(inputs["gview_b"], np.float32)
    uw = np.asarray(inputs["guide_w"], np.float32)
    us = np.asarray(inputs["guide_s"], np.float32)
    ub = np.asarray(inputs["guide_b"], np.float32)

    w11 = np.concatenate([uw * us[:, None, None, None],
                          vw * vs[:, None, None, None],
                          gw * gs[:, None, None, None]], axis=0)  # [11,64,3,3]
    b11 = np.concatenate([ub, vb, gb]).astype(np.float32)
    wpv = np.zeros((128, 9, 32), np.float32)
    for t in range(9):
        di, dj = t // 3, t % 3
        wpv[0:64, t, 0:11] = w11[:, :, di, dj].T
        wpv[64:128, t, 11:22] = w11[:, :, di, dj].T
    with_bias = bool(np.any(b11 != 0.0))
    bpv = np.zeros((1, 32), np.float32)
    bpv[0, 0:11] = b11
    bpv[0, 11:22] = b11

    gfp = np.pad(gfv, ((0, 0), (0, 0), (1, 1), (0, 0)))
    prp = np.pad(prv, ((0, 0), (0, 0), (1, 1), (0, 0)))

    in_maps = []
    for cidx in range(NCORES):
        r0 = cidx * RS
        mmap = {
            "gf": np.ascontiguousarray(gfp[:, :, r0:r0 + RH]),
            "pr": np.ascontiguousarray(prp[:, :, r0:r0 + RH]),
            "mk": np.ascontiguousarray(mkv[:, 0, r0:r0 + RS]),
            "dp": np.ascontiguousarray(dpv[:, :, r0:r0 + RS]),
            "wp": wpv,
        }
        if with_bias:
            mmap["bp"] = bpv
        in_maps.append(mmap)
    return in_maps, with_bias


_EXEC = {}


def _get_exec(with_bias):
    """Build (once) a cached jitted 8-core executable for the program."""
    if with_bias in _EXEC:
        return _EXEC[with_bias]
    if with_bias not in _CACHE:
        _CACHE[with_bias] = _build(with_bias)
    nc = _CACHE[with_bias]

    import jax
    from jax.experimental.shard_map import shard_map
    from jax.sharding import Mesh, PartitionSpec
    from concourse import bass2jax
    from concourse import mybir as mb

    bass2jax.install_neuronx_cc_hook()
    part_name = nc.partition_id_tensor.name if nc.partition_id_tensor else None
    in_names, out_names, out_avals, zero_shapes = [], [], [], []
    for alloc in nc.m.functions[0].allocations:
        if not isinstance(alloc, mb.MemoryLocationSet):
            continue
        name = alloc.memorylocations[0].name
        if alloc.kind == "ExternalInput":
            if name == part_name:
                continue
            in_names.append(name)
        elif alloc.kind == "ExternalOutput":
            out_names.append(name)
            shape = tuple(alloc.tensor_shape)
            dtype = mb.dt.np(alloc.dtype)
            out_avals.append(jax.core.ShapedArray(shape, dtype))
            zero_shapes.append((shape, dtype))
    n_params = len(in_names)
    all_in_names = in_names + out_names
    if part_name is not None:
        all_in_names = all_in_names + [part_name]

    def _body(*args):
        operands = list(args)
        if part_name is not None:
            operands.append(bass2jax.partition_id_tensor())
        outs = bass2jax._bass_exec_p.bind(
            *operands,
            out_avals=tuple(out_avals),
            in_names=tuple(all_in_names),
            out_names=tuple(out_names),
            lowering_input_output_aliases=(),
            sim_require_finite=True,
            sim_require_nnan=True,
            nc=nc,
        )
        return tuple(outs)

    mesh = Mesh(np.asarray(jax.devices()[:NCORES]), ("core",))
    n_outs = len(out_names)
    fn = jax.jit(
        shard_map(_body, mesh=mesh,
                  in_specs=(PartitionSpec("core"),) * (n_params + n_outs),
                  out_specs=(PartitionSpec("core"),) * n_outs,
                  check_rep=False),
        keep_unused=True,
    )
    info = {
        "fn": fn, "in_names": in_names, "out_names": out_names,
        "zero_shapes": zero_shapes, "mesh": mesh,
    }
    _EXEC[with_bias] = info
    return info


def _concat_inputs(info, in_maps):
    return [np.concatenate([m[name] for m in in_maps], axis=0)
            for name in info["in_names"]]


def _zeros(info):
    return [np.zeros((NCORES * s[0],) + tuple(s[1:]), d)
            for s, d in info["zero_shapes"]]


def kernel(**inputs):
    in_maps, with_bias = _prep(inputs)
    info = _get_exec(with_bias)
    out_arrs = info["fn"](*_concat_inputs(info, in_maps), *_zeros(info))
    outs = {}
    for i, name in enumerate(info["out_names"]):
        a = np.asarray(out_arrs[i])
        outs[name] = a.reshape(NCORES, N, C, RS, W)
    out0 = np.concatenate([outs["o0"][c] for c in range(NCORES)], axis=2)
    out1 = np.concatenate([outs["o1"][c] for c in range(NCORES)], axis=2)
    return out0, out1


def time_exec_ns(inputs, reps_a=4, reps_b=16):
    """Marginal per-execution time with device-resident inputs (slope method)."""
    import time as _time
    import jax
    from jax.sharding import NamedSharding, PartitionSpec

    in_maps, with_bias = _prep(inputs)
    info = _get_exec(with_bias)
    sh = NamedSharding(info["mesh"], PartitionSpec("core"))
    dev_in = [jax.device_put(x, sh) for x in _concat_inputs(info, in_maps)]
    dev_z = [jax.device_put(z, sh) for z in _zeros(info)]
    fn = info["fn"]

    out = fn(*dev_in, *dev_z)
    jax.block_until_ready(out)

    def run(k):
        t0 = _time.perf_counter()
        o = None
        for _ in range(k):
            o = fn(*dev_in, *dev_z)
        jax.block_until_ready(o)
        return _time.perf_counter() - t0

    run(2)
    ta = min(run(reps_a) for _ in range(3))
    tb = min(run(reps_b) for _ in range(3))
    slope = (tb - ta) / (reps_b - reps_a)
    return slope * 1e9


def time_exec_loop_ns(inputs, iters=24):
    """On-device timing: run the NEFF `iters` times inside one jax.lax.scan
    dispatch, so per-call host/tunnel overhead is excluded."""
    import time as _time
    import jax
    import jax.numpy as jnp
    from jax.experimental.shard_map import shard_map
    from jax.sharding import Mesh, PartitionSpec, NamedSharding
    from concourse import bass2jax
    from concourse import mybir as mb

    in_maps, with_bias = _prep(inputs)
    info = _get_exec(with_bias)
    nc = _CACHE[with_bias]
    mesh = info["mesh"]
    sh = NamedSharding(mesh, PartitionSpec("core"))
    dev_in = [jax.device_put(x, sh) for x in _concat_inputs(info, in_maps)]
    dev_z = [jax.device_put(z, sh) for z in _zeros(info)]

    part_name = nc.partition_id_tensor.name if nc.partition_id_tensor else None
    in_names = info["in_names"]
    out_names = info["out_names"]
    out_avals = []
    for alloc in nc.m.functions[0].allocations:
        if isinstance(alloc, mb.MemoryLocationSet) and alloc.kind == "ExternalOutput":
            out_avals.append(jax.core.ShapedArray(tuple(alloc.tensor_shape),
                                                  mb.dt.np(alloc.dtype)))
    all_in_names = list(in_names) + list(out_names)
    if part_name is not None:
        all_in_names.append(part_name)

    def _body(*args):
        operands = list(args)
        if part_name is not None:
            operands.append(bass2jax.partition_id_tensor())
        return tuple(bass2jax._bass_exec_p.bind(
            *operands,
            out_avals=tuple(out_avals),
            in_names=tuple(all_in_names),
            out_names=tuple(out_names),
            lowering_input_output_aliases=(),
            sim_require_finite=True,
            sim_require_nnan=True,
            nc=nc,
        ))

    smb = shard_map(_body, mesh=mesh,
                    in_specs=(PartitionSpec("core"),) * (len(in_names) + len(out_names)),
                    out_specs=(PartitionSpec("core"),) * len(out_names),
                    check_rep=False)

    def make_loop(k):
        def loop(ins, zs):
            def step(c, _):
                outs = smb(*ins, *zs)
                return c + outs[0].ravel()[0], None
            c, _ = jax.lax.scan(step, jnp.float32(0.0), None, length=k)
            return c
        return jax.jit(loop, static_argnums=())

    f1 = make_loop(2)
    f2 = make_loop(2 + iters)
    r = f1(dev_in, dev_z); jax.block_until_ready(r)
    r = f2(dev_in, dev_z); jax.block_until_ready(r)

    def run(f):
        t0 = _time.perf_counter()
        jax.block_until_ready(f(dev_in, dev_z))
        return _time.perf_counter() - t0

    t1 = min(run(f1) for _ in range(3))
    t2 = min(run(f2) for _ in range(3))
    return (t2 - t1) / iters * 1e9
